# revision 1
# baseline (speedup 1.0000x reference)
"""Trainium2 Bass kernel for the HexPlane-style decoder (nn_DecoderBase).

Math (B=1): six 3x3 SAME convs (64->16ch) + bias + ReLU + 2x nearest
upsample, channels-last, then broadcast Hadamard into
voxel[t, x, y, z, c] of shape [16, 64, 64, 32, 16] (f32, 128 MiB).

Sharding: X (64) split across 8 cores (8 x-values each; conv halos are
sliced host-side).  Per core the product factorizes as

  out[t,x,y,z,c] = M1[x,y,z,c] * ( ty[t,y,c] * Q[t,x,z,c] ),
  M1 = uxy*uxz*uyz,  Q = utx*utz.

Device layout: partition p = z0*64 + y (z = z0*16 + z1).  ty*Q is computed
by the TensorEngine as K=2 selector matmuls into PSUM (16 matmuls per t,
one per channel), so the VectorEngine runs a single fp32 tensor_tensor
pass (M1 * V) per t, overlapped with the 16 MiB/core output DMA.
"""

import numpy as np

T, XL, Y, Z, C = 16, 8, 64, 32, 16
NCORES = 8
CIN = 64

_CACHE = {}


def _build_program():
    from contextlib import ExitStack

    import concourse.bacc as bacc
    import concourse.bass as bass
    import concourse.mybir as mybir
    from concourse.tile import TileContext

    f32 = mybir.dt.float32
    bf16 = mybir.dt.bfloat16
    AF = mybir.ActivationFunctionType
    MUL = mybir.AluOpType.mult
    AP = bass.AP

    nc = bacc.Bacc()
    ctx = ExitStack()

    # ---- external IO ----
    ein = lambda name, shape: nc.dram_tensor(name, shape, f32, kind="ExternalInput")
    img_xy = ein("img_xy", [CIN, 206])
    img_xz = ein("img_xz", [CIN, 110])
    img_yz = ein("img_yz", [CIN, 614])
    img_tx = ein("img_tx", [CIN, 62])
    img_ty = ein("img_ty", [CIN, 342])
    img_tz = ein("img_tz", [CIN, 182])
    wr = ein("wr", [CIN, 6, 3, 3, 16])      # (cin, plane, dy, dx, cout)
    b_flat = ein("b_flat", [1, 96])
    b_t = ein("b_t", [16, 6])
    ones_in = ein("ones_in", [1, 128])
    out_d = nc.dram_tensor("out", [T, XL, Y, Z, C], f32, kind="ExternalOutput")

    # ---- DRAM scratch: upsampled channels-last conv outputs ----
    dtx = nc.dram_tensor("dtx", [T, XL, C], f32)     # (t, x, c)
    dtz = nc.dram_tensor("dtz", [T, Z, C], f32)      # (t, z, c)
    dty = nc.dram_tensor("dty", [16, 8, 32], f32)    # (c, t', y') pre-upsample
    dxy = nc.dram_tensor("dxy", [XL, Y, C], f32)     # (x, y, c)
    dxz = nc.dram_tensor("dxz", [XL, Z, C], f32)     # (x, z, c)
    dyz = nc.dram_tensor("dyz", [Y, Z, C], f32)      # (y, z, c)
    qd = nc.dram_tensor("qd", [T * XL * Z * C + 16], bf16)  # (t,x,z,c) flat +pad
    # raw conv-output dumps (flat [m*16], junk rows included)
    edump = {k: nc.dram_tensor(f"e_{k}", [m * 16], f32) for k, m in
             [("tx", 48), ("tz0", 72), ("tz1", 72), ("xy0", 68), ("xy1", 68),
              ("xz", 72), ("yz0", 126), ("yz1", 126), ("yz2", 126),
              ("yz3", 126), ("yz4", 72)]}

    with TileContext(nc) as tc:
        sb = lambda name, shape: ctx.enter_context(
            nc.sbuf_tensor(name, shape, f32))
        # inputs
        i_xy, i_xz, i_yz = (sb("i_xy", [CIN, 206]), sb("i_xz", [CIN, 110]),
                            sb("i_yz", [CIN, 614]))
        i_tx, i_ty, i_tz = (sb("i_tx", [CIN, 62]), sb("i_ty", [CIN, 342]),
                            sb("i_tz", [CIN, 182]))
        w_sb, bf_sb = sb("w_sb", [CIN, 864]), sb("bf_sb", [1, 96])
        bt_sb, on_sb = sb("bt_sb", [16, 6]), sb("on_sb", [1, 128])
        # voxel operands
        utx = sb("utx", [128, 16])        # p=(t,x): c
        utz = sb("utz", [128, 512])       # p=(t,x): (z,c)
        q_s = ctx.enter_context(nc.sbuf_tensor("q_s", [128, 512], bf16))
        uxy = sb("uxy", [128, 128])       # p=(z0,y): (x,c)
        uxz = sb("uxz", [128, 2048])      # p=(z0,y): (x,z1,c)
        uyz = sb("uyz", [128, 256])       # p=(z0,y): (z1,c)
        m1a = sb("m1a", [128, 2048])
        m1 = sb("m1", [128, 2048])
        ty_raw = sb("ty_raw", [1, 4096])   # (c', t', y') flat dty copy
        ty_all = ctx.enter_context(nc.sbuf_tensor("ty_all", [1, 8192], bf16))

        # ---------- phase A: input loads (Q-path planes first) ----------
        for dst, srca in ((i_tx, img_tx), (i_tz, img_tz), (i_ty, img_ty)):
            nc.sync.dma_start(dst[:], srca[:])
        nc.sync.dma_start(w_sb[:], wr[:].rearrange("a b c d e -> a (b c d e)"))
        nc.sync.dma_start(bf_sb[:], b_flat[:])
        nc.sync.dma_start(bt_sb[:], b_t[:])
        nc.sync.dma_start(on_sb[:], ones_in[:])
        for dst, srca in ((i_xy, img_xy), (i_xz, img_xz), (i_yz, img_yz)):
            nc.sync.dma_start(dst[:], srca[:])

        # ---------- phase B: convolutions + staging, Q-path first ----------
        def wslice(i, dy, dx):
            off = ((i * 3 + dy) * 3 + dx) * 16
            return w_sb[:, off:off + 16]

        conv_pool_cm = tc.tile_pool(name="convpsum", bufs=2, space="PSUM")
        conv_pool = conv_pool_cm.__enter__()

        conv_outs = {}

        def conv_spatial(i, img, fsz, wp, rows, row0, tag):
            # Full-width contiguous windows (stationary AP must be 1-D):
            # out flat m = r*wp + col; junk at cols wp-2, wp-1.
            m = rows * wp
            psum = conv_pool.tile([m, 16], f32, name=f"cp_{tag}", tag="cp")
            for dy in range(3):
                for dx in range(3):
                    lhsT = AP(img, (row0 + dy) * wp + dx, [[fsz, CIN], [1, m]])
                    nc.tensor.matmul(psum, lhsT, wslice(i, dy, dx),
                                     start=(dy == 0 and dx == 0), stop=False)
            nc.tensor.matmul(psum, on_sb[:, :m],
                             bf_sb[:, i * 16:i * 16 + 16], start=False, stop=True)
            out_sb = sb(f"c_{tag}", [m, 16])
            nc.scalar.activation(out_sb[:], psum, AF.Relu)
            conv_outs[tag] = out_sb

        dump_insts = {}
        stage_insts = {}

        def dump(eng, k):
            dump_insts[k] = eng.dma_start(edump[k][:], conv_outs[k][:])

        def stage(eng, key, srck, dst_ap, src_ap):
            inst = eng.dma_start(dst_ap, src_ap)
            if srck is not None:
                bass._add_dep_helper(inst.ins, dump_insts[srck].ins,
                                     reason=f"dump {srck}")
            stage_insts.setdefault(key, []).append(inst)

        def after_stage(key, inst):
            for si in stage_insts[key]:
                bass._add_dep_helper(inst.ins, si.ins, reason=f"raw {key}")
            return inst

        # --- tx ---
        conv_spatial(3, i_tx, 62, 6, 8, 0, "tx")              # m=48
        dump(nc.sync, "tx")
        for rt in range(2):
            for rx in range(2):
                stage(nc.sync, "dtx", "tx",
                      AP(dtx, rt * XL * C + rx * C,
                         [[2 * XL * C, 8], [2 * C, 4], [1, 16]]),
                      AP(edump["tx"], 0, [[96, 8], [16, 4], [1, 16]]))
        # --- tz ---
        conv_spatial(5, i_tz, 182, 18, 4, 0, "tz0")           # m=72
        conv_spatial(5, i_tz, 182, 18, 4, 4, "tz1")
        for k in range(2):
            dump(nc.sync, f"tz{k}")
            for rt in range(2):
                for rz in range(2):
                    stage(nc.sync, "dtz", f"tz{k}",
                          AP(dtz, (8 * k + rt) * Z * C + rz * C,
                             [[2 * Z * C, 4], [2 * C, 16], [1, 16]]),
                          AP(edump[f"tz{k}"], 0, [[288, 4], [16, 16], [1, 16]]))
        # --- ty (cout-partition conv; W stationary) ---
        psum_ty = conv_pool.tile([16, 272], f32, name="cp_ty", tag="cpty")
        for dy in range(3):
            for dx in range(3):
                rhs = AP(i_ty, dy * 34 + dx, [[342, CIN], [1, 272]])
                nc.tensor.matmul(psum_ty, wslice(4, dy, dx), rhs,
                                 start=(dy == 0 and dx == 0),
                                 stop=(dy == 2 and dx == 2))
        cty = sb("cty", [16, 272])
        nc.scalar.activation(cty[:], psum_ty, AF.Relu, bias=bt_sb[:, 4:5])
        stage_insts["dty"] = [nc.sync.dma_start(
            dty[:].rearrange("a b c -> a (b c)"),
            AP(cty, 0, [[272, 16], [34, 8], [1, 32]]))]

        # --- Q = utx * utz ---
        after_stage("dtx", nc.sync.dma_start(
            utx[:], AP(dtx, 0, [[16, 128], [1, 16]])))
        after_stage("dtz", nc.sync.dma_start(
            utz[:], AP(dtz, 0, [[Z * C, 16], [0, 8], [1, Z * C]])))
        nc.vector.tensor_tensor(
            q_s[:], utz[:], AP(utx, 0, [[16, 128], [0, 32], [1, 16]]), MUL)
        q_store = nc.sync.dma_start(AP(qd, 0, [[512, 128], [1, 512]]), q_s[:])

        # --- ty_all strip (vector copies; y upsampled, cast to bf16) ---
        after_stage("dty", nc.sync.dma_start(ty_raw[:], dty[:]))
        for tp in range(8):
            nc.vector.tensor_copy(
                AP(ty_all, tp * 1024, [[8192, 1], [1, 1024]]),
                AP(ty_raw, tp * 32, [[4096, 1], [256, 16], [1, 32], [0, 2]]))

        # --- M1-path planes (staging DMAs on the scalar HWDGE engine) ---
        conv_spatial(0, i_xy, 206, 34, 2, 0, "xy0")           # m=68
        conv_spatial(0, i_xy, 206, 34, 2, 2, "xy1")
        for k in range(2):
            dump(nc.scalar, f"xy{k}")
            for rx in range(2):
                for ry in range(2):
                    stage(nc.scalar, "dxy", f"xy{k}",
                          AP(dxy, (4 * k + rx) * Y * C + ry * C,
                             [[2 * Y * C, 2], [2 * C, 32], [1, 16]]),
                          AP(edump[f"xy{k}"], 0, [[544, 2], [16, 32], [1, 16]]))
        conv_spatial(1, i_xz, 110, 18, 4, 0, "xz")            # m=72
        dump(nc.scalar, "xz")
        for rx in range(2):
            for rz in range(2):
                stage(nc.scalar, "dxz", "xz",
                      AP(dxz, rx * Z * C + rz * C,
                         [[2 * Z * C, 4], [2 * C, 16], [1, 16]]),
                      AP(edump["xz"], 0, [[288, 4], [16, 16], [1, 16]]))
        yz_rows = [(0, 7), (7, 7), (14, 7), (21, 7), (28, 4)]
        for bb, (r0, nr) in enumerate(yz_rows):
            conv_spatial(2, i_yz, 614, 18, nr, r0, f"yz{bb}")
            dump(nc.scalar, f"yz{bb}")
            for ry in range(2):
                for rz in range(2):
                    stage(nc.scalar, "dyz", f"yz{bb}",
                          AP(dyz, (2 * r0 + ry) * Z * C + rz * C,
                             [[2 * Z * C, nr], [2 * C, 16], [1, 16]]),
                          AP(edump[f"yz{bb}"], 0, [[288, nr], [16, 16], [1, 16]]))

        conv_pool_cm.__exit__(None, None, None)

        # ---------- M1 operand loads + build ----------
        for z0 in range(2):
            after_stage("dxy", nc.scalar.dma_start(
                uxy[z0 * 64:(z0 + 1) * 64, :],
                AP(dxy, 0, [[C, 64], [Y * C, 8], [1, 16]])))
            after_stage("dxz", nc.scalar.dma_start(
                uxz[z0 * 64:(z0 + 1) * 64, :],
                AP(dxz, z0 * 16 * C, [[0, 64], [Z * C, 8], [1, 256]])))
        after_stage("dyz", nc.scalar.dma_start(
            uyz[:], AP(dyz, 0, [[16 * C, 2], [Z * C, 64], [1, 256]])))

        nc.vector.tensor_tensor(
            m1a[:], uxz[:], AP(uyz, 0, [[256, 128], [0, 8], [1, 256]]), MUL)
        nc.vector.tensor_tensor(
            m1[:], m1a[:], AP(uxy, 0, [[128, 128], [16, 8], [0, 16], [1, 16]]),
            MUL)

        # ---------- phase E: per-t voxel ----------
        from contextlib import ExitStack as _ES
        pool_ctx = _ES()
        qz_pool = pool_ctx.enter_context(tc.tile_pool(name="qz", bufs=1))
        v_pool = pool_ctx.enter_context(
            tc.tile_pool(name="vps", bufs=2, space="PSUM"))
        out_pool = pool_ctx.enter_context(tc.tile_pool(name="outsb", bufs=3))

        for tg in range(4):
            # Q rows for 4 t's x 2 z0-halves; "o" copies shifted one element
            # so odd-channel slices read 4-byte-aligned bases
            qzh, qzo = [], []
            for z0 in range(2):
                qz = qz_pool.tile([1, 8192], bf16, name=f"qz{z0}", tag=f"qz{z0}")
                bass._add_dep_helper(
                    nc.sync.dma_start(
                        qz, AP(qd, tg * 4 * XL * Z * C + z0 * 16 * C,
                               [[XL * Z * C, 4], [Z * C, 8], [1, 256]])).ins,
                    q_store.ins, reason="raw qd")
                qzh.append(qz)
                qo = qz_pool.tile([1, 8192], bf16, name=f"qo{z0}", tag=f"qo{z0}")
                bass._add_dep_helper(
                    nc.sync.dma_start(
                        qo, AP(qd, tg * 4 * XL * Z * C + z0 * 16 * C + 1,
                               [[XL * Z * C, 4], [Z * C, 8], [1, 256]])).ins,
                    q_store.ins, reason="raw qd")
                qzo.append(qo)

            for ti in range(4):
                t = tg * 4 + ti
                v = v_pool.tile([128, 2048], f32, name="v", tag="v")
                vp = v.ap[0][0]
                for z0 in range(2):
                    for cp in range(16):
                        lhsT = AP(ty_all, (t // 2) * 1024 + cp * 64,
                                  [[8192, 1], [1, 64]])
                        if cp % 2 == 0:
                            rhs = AP(qzh[z0].tensor,
                                     qzh[z0].offset + ti * 2048 + cp,
                                     [[8192, 1], [256, 8], [16, 16]])
                        else:
                            rhs = AP(qzo[z0].tensor,
                                     qzo[z0].offset + ti * 2048 + cp - 1,
                                     [[8192, 1], [256, 8], [16, 16]])
                        nc.tensor.matmul(
                            v[z0 * 64:(z0 + 1) * 64, cp * 128:(cp + 1) * 128],
                            lhsT, rhs, start=True, stop=True)

                o = out_pool.tile([128, 2048], f32, name="o", tag="o")
                op = o.ap[0][0]
                nc.vector.tensor_tensor(
                    AP(o.tensor, o.offset,
                       [[op, 128], [256, 8], [16, 16], [1, 16]]),
                    AP(m1, 0, [[2048, 128], [256, 8], [16, 16], [1, 16]]),
                    AP(v.tensor, v.offset,
                       [[vp, 128], [16, 8], [1, 16], [128, 16]]),
                    MUL)
                for z0 in range(2):
                    dst = AP(out_d, t * XL * Y * Z * C + z0 * 16 * C,
                             [[Z * C, 64], [Y * Z * C, 8], [1, 256]])
                    nc.scalar.dma_start(dst, o[z0 * 64:(z0 + 1) * 64, :])

        pool_ctx.close()

    nc.compile()
    return nc, ctx


def _prep_inputs(plane_xy, plane_xz, plane_yz, plane_tx, plane_ty, plane_tz, W, b):
    """Host-side slicing/padding/transposition. Returns per-core input maps."""
    f32 = np.float32
    xy = np.asarray(plane_xy, f32)[0]  # [64, X'32, Y'32]
    xz = np.asarray(plane_xz, f32)[0]  # [64, X'32, Z'16]
    yz = np.asarray(plane_yz, f32)[0]  # [64, Y'32, Z'16]
    tx = np.asarray(plane_tx, f32)[0]  # [64, T'8,  X'32]
    ty = np.asarray(plane_ty, f32)[0]  # [64, T'8,  Y'32]
    tz = np.asarray(plane_tz, f32)[0]  # [64, T'8,  Z'16]
    W = np.asarray(W, f32)             # [6, 16, 64, 3, 3]
    b = np.asarray(b, f32)             # [6, 16]

    wr = np.ascontiguousarray(W.transpose(2, 0, 3, 4, 1))  # (ci, i, dy, dx, co)
    b_flat = np.ascontiguousarray(b.reshape(1, 96))
    b_t = np.ascontiguousarray(b.T)
    ones = np.ones((1, 128), f32)

    def flat2(p):
        q = p.reshape(p.shape[0], -1)
        return np.ascontiguousarray(
            np.pad(q, ((0, 0), (0, 2))))

    img_yz = flat2(np.pad(yz, ((0, 0), (1, 1), (1, 1))))
    img_ty = flat2(np.pad(ty, ((0, 0), (1, 1), (1, 1))))
    img_tz = flat2(np.pad(tz, ((0, 0), (1, 1), (1, 1))))

    def row_halo(p, x0h):
        out = np.zeros((p.shape[0], 6, p.shape[2]), f32)
        lo = x0h - 1
        s0, s1 = max(lo, 0), min(lo + 6, p.shape[1])
        out[:, s0 - lo:s0 - lo + (s1 - s0), :] = p[:, s0:s1, :]
        return out

    def col_halo(p, x0h):
        out = np.zeros((p.shape[0], p.shape[1], 6), f32)
        lo = x0h - 1
        s0, s1 = max(lo, 0), min(lo + 6, p.shape[2])
        out[:, :, s0 - lo:s0 - lo + (s1 - s0)] = p[:, :, s0:s1]
        return out

    in_maps = []
    for k in range(NCORES):
        x0h = 4 * k
        in_maps.append({
            "img_xy": flat2(np.pad(row_halo(xy, x0h), ((0, 0), (0, 0), (1, 1)))),
            "img_xz": flat2(np.pad(row_halo(xz, x0h), ((0, 0), (0, 0), (1, 1)))),
            "img_yz": img_yz,
            "img_tx": flat2(np.pad(col_halo(tx, x0h), ((0, 0), (1, 1), (0, 0)))),
            "img_ty": img_ty,
            "img_tz": img_tz,
            "wr": wr,
            "b_flat": b_flat,
            "b_t": b_t,
            "ones_in": ones,
        })
    return in_maps


def kernel(plane_xy, plane_xz, plane_yz, plane_tx, plane_ty, plane_tz, W, b):
    from concourse.bass_utils import run_bass_kernel_spmd

    if "nc" not in _CACHE:
        _CACHE["nc"], _CACHE["ctx"] = _build_program()
    nc = _CACHE["nc"]

    in_maps = _prep_inputs(plane_xy, plane_xz, plane_yz, plane_tx, plane_ty,
                           plane_tz, W, b)
    res = run_bass_kernel_spmd(nc, in_maps, list(range(NCORES)))
    slices = [res.results[k]["out"] for k in range(NCORES)]
    full = np.concatenate(slices, axis=1)  # [T, 64, Y, Z, C]
    return full[None].astype(np.float32)



# revision 25
# speedup vs baseline: 1.4530x; 1.4530x over previous
"""Trainium2 Bass kernel for the HexPlane-style decoder (nn_DecoderBase).

Math (B=1): six 3x3 SAME convs (64->16ch) + bias + ReLU + 2x nearest
upsample, channels-last, then broadcast Hadamard into
voxel[t, x, y, z, c] of shape [16, 64, 64, 32, 16] (128 MiB f32).

Sharding: X (64) split across 8 cores (8 x-values each; conv halos are
sliced host-side).  Per core the product factorizes as

  out[t,x,y,z,c] = M1[x,y,z,c] * TY[t,y,c] * Q[t,x,z,c],
  M1 = uxy*uxz*uyz,  Q = utx*utz.

Device layout: partition p = z0*64 + y (z = z0*16 + z1), free = (x,z1,c).
Per t: the TensorEngine broadcasts Q across all 128 partitions with a
single K=2 "z0-selector" matmul per PSUM bank (4 matmuls of N=512); the
Scalar engine evicts PSUM->SBUF as bf16; the Vector engine runs two
all-bf16 tensor_tensor passes (M1*TY_t, then *Qbc) at 2x DVE rate; the
output is stored as bf16 (host widens to f32) with fully-contiguous
4 KiB-per-partition DMA descriptors, one 512 KiB DMA per t.
"""

import numpy as np

T, XL, Y, Z, C = 16, 8, 64, 32, 16
NCORES = 8
CIN = 64

_CACHE = {}


def _build_program():
    from contextlib import ExitStack

    import concourse.bacc as bacc
    import concourse.bass as bass
    import concourse.mybir as mybir
    from concourse.tile import TileContext

    f32 = mybir.dt.float32
    bf16 = mybir.dt.bfloat16
    AF = mybir.ActivationFunctionType
    MUL = mybir.AluOpType.mult
    AP = bass.AP

    nc = bacc.Bacc()
    ctx = ExitStack()

    # ---- external IO ----
    ein = lambda name, shape: nc.dram_tensor(name, shape, f32, kind="ExternalInput")
    img_xy = ein("img_xy", [CIN, 206])
    img_xz = ein("img_xz", [CIN, 110])
    img_yz = ein("img_yz", [CIN, 614])
    img_tx = ein("img_tx", [CIN, 62])
    img_ty = ein("img_ty", [CIN, 342])
    img_tz = ein("img_tz", [CIN, 182])
    wr = ein("wr", [CIN, 6, 3, 3, 16])      # (cin, plane, dy, dx, cout)
    b_flat = ein("b_flat", [1, 96])
    ones_in = ein("ones_in", [1, 128])
    sel_in = ein("sel_in", [2, 128])
    # out layout: [t, p=(z0,y), f=(x,z1,c)] bf16; host transposes/widens.
    out_d = nc.dram_tensor("out", [T, 128, 2048], bf16, kind="ExternalOutput")

    # ---- DRAM scratch: upsampled channels-last conv outputs ----
    dtx = nc.dram_tensor("dtx", [T, XL, C], f32)     # (t, x, c)
    dtz = nc.dram_tensor("dtz", [T, Z, C], f32)      # (t, z, c)
    dty = nc.dram_tensor("dty", [8, 64, 16], bf16)   # (t', y, c), y upsampled
    dxy = nc.dram_tensor("dxy", [XL, Y, C], f32)     # (x, y, c)
    dxz = nc.dram_tensor("dxz", [XL, Z, C], f32)     # (x, z, c)
    dyz = nc.dram_tensor("dyz", [Y, Z, C], f32)      # (y, z, c)
    # raw conv-output dumps (flat [m*16], junk rows included)
    edump = {k: nc.dram_tensor(f"e_{k}", [m * 16], f32) for k, m in
             [("tx", 48), ("tz0", 72), ("tz1", 72), ("xy0", 68), ("xy1", 68),
              ("xz", 72), ("yz0", 126), ("yz1", 126), ("yz2", 126),
              ("yz3", 126), ("yz4", 72)]}
    for k, m in [("ty0", 102), ("ty1", 102), ("ty2", 68)]:
        edump[k] = nc.dram_tensor(f"e_{k}", [m * 16], bf16)

    with TileContext(nc) as tc:
        sb = lambda name, shape, dt=f32: ctx.enter_context(
            nc.sbuf_tensor(name, shape, dt))
        # inputs
        i_xy, i_xz, i_yz = (sb("i_xy", [CIN, 206]), sb("i_xz", [CIN, 110]),
                            sb("i_yz", [CIN, 614]))
        i_tx, i_ty, i_tz = (sb("i_tx", [CIN, 62]), sb("i_ty", [CIN, 342]),
                            sb("i_tz", [CIN, 182]))
        w_sb, bf_sb = sb("w_sb", [CIN, 864]), sb("bf_sb", [1, 96])
        on_sb = sb("on_sb", [1, 128])
        sel_f = sb("sel_f", [2, 128])
        sel_b = sb("sel_b", [2, 128], bf16)
        # voxel operands
        utx3 = sb("utx3", [32, 128])       # p=(z0,t): (x,c)
        utz3 = sb("utz3", [32, 256])       # p=(z0,t): (z1,c)
        q32 = sb("q32", [32, 2048], bf16)  # p=(z0,t): (x,z1,c)
        q_all = sb("q_all", [2, 32768], bf16)   # p=z0: (t,x,z1,c)
        TYs = sb("TYs", [128, 128], bf16)  # p=(z0,y): (t',c)
        uxy = sb("uxy", [128, 128])        # p=(z0,y): (x,c)
        uxz = sb("uxz", [128, 2048])       # p=(z0,y): (x,z1,c)
        uyz = sb("uyz", [128, 256])        # p=(z0,y): (z1,c)
        m1a = sb("m1a", [128, 2048])
        m1 = sb("m1", [128, 2048], bf16)

        # ---------- phase A: input loads (Q-path planes first) ----------
        for dst, srca in ((i_tx, img_tx), (i_tz, img_tz), (i_ty, img_ty)):
            nc.sync.dma_start(dst[:], srca[:])
        nc.sync.dma_start(w_sb[:], wr[:].rearrange("a b c d e -> a (b c d e)"))
        nc.sync.dma_start(bf_sb[:], b_flat[:])
        nc.sync.dma_start(on_sb[:], ones_in[:])
        nc.sync.dma_start(sel_f[:], sel_in[:])
        for dst, srca in ((i_xy, img_xy), (i_xz, img_xz), (i_yz, img_yz)):
            nc.sync.dma_start(dst[:], srca[:])
        nc.vector.tensor_copy(sel_b[:], sel_f[:])

        # ---------- phase B: convolutions + staging, Q-path first ----------
        def wslice(i, dy, dx):
            off = ((i * 3 + dy) * 3 + dx) * 16
            return w_sb[:, off:off + 16]

        conv_pool_cm = tc.tile_pool(name="convpsum", bufs=2, space="PSUM")
        conv_pool = conv_pool_cm.__enter__()

        conv_outs = {}

        def conv_spatial(i, img, fsz, wp, rows, row0, tag, dt=f32):
            # Full-width contiguous windows (stationary AP must be 1-D):
            # out flat m = r*wp + col; junk at cols wp-2, wp-1.
            m = rows * wp
            psum = conv_pool.tile([m, 16], f32, name=f"cp_{tag}", tag="cp")
            for dy in range(3):
                for dx in range(3):
                    lhsT = AP(img, (row0 + dy) * wp + dx, [[fsz, CIN], [1, m]])
                    nc.tensor.matmul(psum, lhsT, wslice(i, dy, dx),
                                     start=(dy == 0 and dx == 0), stop=False)
            nc.tensor.matmul(psum, on_sb[:, :m],
                             bf_sb[:, i * 16:i * 16 + 16], start=False, stop=True)
            out_sb = sb(f"c_{tag}", [m, 16], dt)
            nc.scalar.activation(out_sb[:], psum, AF.Relu)
            conv_outs[tag] = out_sb

        dump_insts = {}
        stage_insts = {}

        def dump(eng, k):
            dump_insts[k] = eng.dma_start(edump[k][:], conv_outs[k][:])

        def stage(eng, key, srck, dst_ap, src_ap):
            inst = eng.dma_start(dst_ap, src_ap)
            if srck is not None:
                bass._add_dep_helper(inst.ins, dump_insts[srck].ins,
                                     reason=f"dump {srck}")
            stage_insts.setdefault(key, []).append(inst)

        def after_stage(key, inst):
            for si in stage_insts[key]:
                bass._add_dep_helper(inst.ins, si.ins, reason=f"raw {key}")
            return inst

        # --- tx ---
        conv_spatial(3, i_tx, 62, 6, 8, 0, "tx")              # m=48
        dump(nc.sync, "tx")
        for rt in range(2):
            for rx in range(2):
                stage(nc.sync, "dtx", "tx",
                      AP(dtx, rt * XL * C + rx * C,
                         [[2 * XL * C, 8], [2 * C, 4], [1, 16]]),
                      AP(edump["tx"], 0, [[96, 8], [16, 4], [1, 16]]))
        # --- tz ---
        conv_spatial(5, i_tz, 182, 18, 4, 0, "tz0")           # m=72
        conv_spatial(5, i_tz, 182, 18, 4, 4, "tz1")
        for k in range(2):
            dump(nc.sync, f"tz{k}")
            for rt in range(2):
                for rz in range(2):
                    stage(nc.sync, "dtz", f"tz{k}",
                          AP(dtz, (8 * k + rt) * Z * C + rz * C,
                             [[2 * Z * C, 4], [2 * C, 16], [1, 16]]),
                          AP(edump[f"tz{k}"], 0, [[288, 4], [16, 16], [1, 16]]))
        # --- ty (position-partition conv like the others; (t',y,c) bf16,
        # y nearest-up2 done by staging each y' row twice) ---
        ty_rows = [(0, 3), (3, 3), (6, 2)]
        for bb, (r0, nr) in enumerate(ty_rows):
            conv_spatial(4, i_ty, 342, 34, nr, r0, f"ty{bb}", dt=bf16)
            dump(nc.sync, f"ty{bb}")
            for yd in range(2):
                stage(nc.sync, "dty", f"ty{bb}",
                      AP(dty, r0 * 1024 + yd * 16,
                         [[1024, nr], [32, 32], [1, 16]]),
                      AP(edump[f"ty{bb}"], 0, [[544, nr], [16, 32], [1, 16]]))

        # --- Q = utx * utz  (q_all[z0, (t,x,z1,c)] for matmul rhs) ---
        for z0 in range(2):
            after_stage("dtx", nc.sync.dma_start(
                AP(utx3, z0 * 16 * 128, [[128, 16], [1, 128]]),
                AP(dtx, 0, [[128, 16], [1, 128]])))
            after_stage("dtz", nc.sync.dma_start(
                AP(utz3, z0 * 16 * 256, [[256, 16], [1, 256]]),
                AP(dtz, z0 * 256, [[512, 16], [1, 256]])))
        nc.vector.tensor_tensor(
            q32[:], AP(utz3, 0, [[256, 32], [0, 8], [1, 256]]),
            AP(utx3, 0, [[128, 32], [16, 8], [0, 16], [1, 16]]), MUL)
        nc.sync.dma_start(
            AP(q_all, 0, [[32768, 2], [2048, 16], [1, 2048]]), q32[:])

        # --- TY strip [p=(z0,y), (t',c)] bf16; y-dup already in dty,
        # t upsampled later by reading offset (t//2)*16 in pass1 ---
        for z0 in range(2):
            after_stage("dty", nc.sync.dma_start(
                AP(TYs, z0 * 64 * 128, [[128, 64], [1, 128]]),
                AP(dty, 0, [[16, 64], [1024, 8], [1, 16]])))

        # --- M1-path planes (staging DMAs on the scalar HWDGE engine) ---
        conv_spatial(0, i_xy, 206, 34, 2, 0, "xy0")           # m=68
        conv_spatial(0, i_xy, 206, 34, 2, 2, "xy1")
        for k in range(2):
            dump(nc.scalar, f"xy{k}")
            for rx in range(2):
                for ry in range(2):
                    stage(nc.scalar, "dxy", f"xy{k}",
                          AP(dxy, (4 * k + rx) * Y * C + ry * C,
                             [[2 * Y * C, 2], [2 * C, 32], [1, 16]]),
                          AP(edump[f"xy{k}"], 0, [[544, 2], [16, 32], [1, 16]]))
        conv_spatial(1, i_xz, 110, 18, 4, 0, "xz")            # m=72
        dump(nc.scalar, "xz")
        for rx in range(2):
            for rz in range(2):
                stage(nc.scalar, "dxz", "xz",
                      AP(dxz, rx * Z * C + rz * C,
                         [[2 * Z * C, 4], [2 * C, 16], [1, 16]]),
                      AP(edump["xz"], 0, [[288, 4], [16, 16], [1, 16]]))
        yz_rows = [(0, 7), (7, 7), (14, 7), (21, 7), (28, 4)]
        for bb, (r0, nr) in enumerate(yz_rows):
            conv_spatial(2, i_yz, 614, 18, nr, r0, f"yz{bb}")
            dump(nc.scalar, f"yz{bb}")
            for ry in range(2):
                for rz in range(2):
                    stage(nc.scalar, "dyz", f"yz{bb}",
                          AP(dyz, (2 * r0 + ry) * Z * C + rz * C,
                             [[2 * Z * C, nr], [2 * C, 16], [1, 16]]),
                          AP(edump[f"yz{bb}"], 0, [[288, nr], [16, 16], [1, 16]]))

        conv_pool_cm.__exit__(None, None, None)

        # ---------- M1 operand loads + build (m1 in bf16) ----------
        for z0 in range(2):
            after_stage("dxy", nc.scalar.dma_start(
                uxy[z0 * 64:(z0 + 1) * 64, :],
                AP(dxy, 0, [[C, 64], [Y * C, 8], [1, 16]])))
            after_stage("dxz", nc.scalar.dma_start(
                uxz[z0 * 64:(z0 + 1) * 64, :],
                AP(dxz, z0 * 16 * C, [[0, 64], [Z * C, 8], [1, 256]])))
        after_stage("dyz", nc.scalar.dma_start(
            uyz[:], AP(dyz, 0, [[16 * C, 2], [Z * C, 64], [1, 256]])))

        nc.vector.tensor_tensor(
            m1a[:], uxz[:], AP(uyz, 0, [[256, 128], [0, 8], [1, 256]]), MUL)
        nc.vector.tensor_tensor(
            m1[:], m1a[:], AP(uxy, 0, [[128, 128], [16, 8], [0, 16], [1, 16]]),
            MUL)

        # ---------- phase E: per-t voxel ----------
        from contextlib import ExitStack as _ES
        pool_ctx = _ES()
        v_pool = pool_ctx.enter_context(
            tc.tile_pool(name="vps", bufs=2, space="PSUM"))
        qbc_pool = pool_ctx.enter_context(tc.tile_pool(name="qbc", bufs=2))
        m1t_pool = pool_ctx.enter_context(tc.tile_pool(name="m1t", bufs=2))
        out_pool = pool_ctx.enter_context(tc.tile_pool(name="outsb", bufs=3))

        for t in range(T):
            # PE: broadcast Q_t across all 128 partitions via K=2 selector
            v = v_pool.tile([128, 2048], f32, name="v", tag="v")
            for bank in range(4):
                rhs = AP(q_all, t * 2048 + bank * 512, [[32768, 2], [1, 512]])
                nc.tensor.matmul(v[:, bank * 512:(bank + 1) * 512],
                                 sel_b[:], rhs, start=True, stop=True)
            # Scalar: evict PSUM -> SBUF bf16
            qbc = qbc_pool.tile([128, 2048], bf16, name="qbc", tag="qbc")
            nc.scalar.activation(qbc[:], v[:], AF.Copy)
            # DVE: m1t = m1 * TY_t   (all-bf16, 2x rate)
            m1t = m1t_pool.tile([128, 2048], bf16, name="m1t", tag="m1t")
            nc.vector.tensor_tensor(
                m1t[:], m1[:],
                AP(TYs, (t // 2) * 16, [[128, 128], [0, 8], [0, 16], [1, 16]]),
                MUL)
            # DVE: o = m1t * qbc    (all-bf16, 2x rate)
            o = out_pool.tile([128, 2048], bf16, name="o", tag="o")
            nc.vector.tensor_tensor(o[:], m1t[:], qbc[:], MUL)
            # store: fully contiguous 4 KiB per partition
            nc.sync.dma_start(
                AP(out_d, t * 128 * 2048, [[2048, 128], [1, 2048]]), o[:])

        pool_ctx.close()

    nc.compile()
    return nc, ctx


def _prep_inputs(plane_xy, plane_xz, plane_yz, plane_tx, plane_ty, plane_tz, W, b):
    """Host-side slicing/padding/transposition. Returns per-core input maps."""
    f32 = np.float32
    xy = np.asarray(plane_xy, f32)[0]  # [64, X'32, Y'32]
    xz = np.asarray(plane_xz, f32)[0]  # [64, X'32, Z'16]
    yz = np.asarray(plane_yz, f32)[0]  # [64, Y'32, Z'16]
    tx = np.asarray(plane_tx, f32)[0]  # [64, T'8,  X'32]
    ty = np.asarray(plane_ty, f32)[0]  # [64, T'8,  Y'32]
    tz = np.asarray(plane_tz, f32)[0]  # [64, T'8,  Z'16]
    W = np.asarray(W, f32)             # [6, 16, 64, 3, 3]
    b = np.asarray(b, f32)             # [6, 16]

    wr = np.ascontiguousarray(W.transpose(2, 0, 3, 4, 1))  # (ci, i, dy, dx, co)
    b_flat = np.ascontiguousarray(b.reshape(1, 96))
    ones = np.ones((1, 128), f32)
    sel = np.zeros((2, 128), f32)
    sel[0, :64] = 1.0
    sel[1, 64:] = 1.0

    def flat2(p):
        q = p.reshape(p.shape[0], -1)
        return np.ascontiguousarray(
            np.pad(q, ((0, 0), (0, 2))))

    img_yz = flat2(np.pad(yz, ((0, 0), (1, 1), (1, 1))))
    img_ty = flat2(np.pad(ty, ((0, 0), (1, 1), (1, 1))))
    img_tz = flat2(np.pad(tz, ((0, 0), (1, 1), (1, 1))))

    def row_halo(p, x0h):
        out = np.zeros((p.shape[0], 6, p.shape[2]), f32)
        lo = x0h - 1
        s0, s1 = max(lo, 0), min(lo + 6, p.shape[1])
        out[:, s0 - lo:s0 - lo + (s1 - s0), :] = p[:, s0:s1, :]
        return out

    def col_halo(p, x0h):
        out = np.zeros((p.shape[0], p.shape[1], 6), f32)
        lo = x0h - 1
        s0, s1 = max(lo, 0), min(lo + 6, p.shape[2])
        out[:, :, s0 - lo:s0 - lo + (s1 - s0)] = p[:, :, s0:s1]
        return out

    in_maps = []
    for k in range(NCORES):
        x0h = 4 * k
        in_maps.append({
            "img_xy": flat2(np.pad(row_halo(xy, x0h), ((0, 0), (0, 0), (1, 1)))),
            "img_xz": flat2(np.pad(row_halo(xz, x0h), ((0, 0), (0, 0), (1, 1)))),
            "img_yz": img_yz,
            "img_tx": flat2(np.pad(col_halo(tx, x0h), ((0, 0), (1, 1), (0, 0)))),
            "img_ty": img_ty,
            "img_tz": img_tz,
            "wr": wr,
            "b_flat": b_flat,
            "ones_in": ones,
            "sel_in": sel,
        })
    return in_maps


def _unshard(out_np):
    # device out: [T, p=(z0,y), f=(x,z1,c)] -> [T, XL, Y, Z, C] f32
    o = np.asarray(out_np).reshape(T, 2, Y, XL, 16, C)
    o = o.transpose(0, 3, 2, 1, 4, 5).reshape(T, XL, Y, Z, C)
    return o.astype(np.float32)


def kernel(plane_xy, plane_xz, plane_yz, plane_tx, plane_ty, plane_tz, W, b):
    from concourse.bass_utils import run_bass_kernel_spmd

    if "nc" not in _CACHE:
        _CACHE["nc"], _CACHE["ctx"] = _build_program()
    nc = _CACHE["nc"]

    in_maps = _prep_inputs(plane_xy, plane_xz, plane_yz, plane_tx, plane_ty,
                           plane_tz, W, b)
    res = run_bass_kernel_spmd(nc, in_maps, list(range(NCORES)))
    slices = [_unshard(res.results[k]["out"]) for k in range(NCORES)]
    full = np.concatenate(slices, axis=1)  # [T, 64, Y, Z, C]
    return full[None].astype(np.float32)


# revision 27
# speedup vs baseline: 2.0512x; 1.4117x over previous
"""Trainium2 Bass kernel for the HexPlane-style decoder (nn_DecoderBase).

Math (B=1): six 3x3 SAME convs (64->16ch) + bias + ReLU + 2x nearest
upsample, channels-last, then broadcast Hadamard into
voxel[t, x, y, z, c] of shape [16, 64, 64, 32, 16] (128 MiB f32).

Sharding: X (64) split across 8 cores (8 x-values each; conv halos are
sliced host-side).  Per core the product factorizes (reassociated) as

  out[t,x,y,z,c] = M1'[x,y,z,c] * ( TY[t,y,c] * Q2[t,x,z,c] ),
  M1' = uxy*uyz,  Q2 = utx*utz*uxz   (xz folded into the Q side).

Device layout: partition p = z0*64 + y (z = z0*16 + z1), free = (x,z1,c).
Per t: TensorEngine broadcasts Q2 across all 128 partitions with K=2
"z0-selector" matmuls (4 banks of N=512, bf16); Scalar evicts PSUM->SBUF
bf16; DVE runs two all-bf16 tensor_tensor passes (TY_t*Qbc, then *M1')
at 2x rate; output stored bf16 (host widens) with fully-contiguous
4 KiB-per-partition descriptors, one 512 KiB DMA per t on the sync queue.
Convs run in bf16 (host pre-casts inputs).
"""

import numpy as np

T, XL, Y, Z, C = 16, 8, 64, 32, 16
NCORES = 8
CIN = 64

_CACHE = {}


def _build_program():
    from contextlib import ExitStack

    import concourse.bacc as bacc
    import concourse.bass as bass
    import concourse.mybir as mybir
    from concourse.tile import TileContext

    f32 = mybir.dt.float32
    bf16 = mybir.dt.bfloat16
    AF = mybir.ActivationFunctionType
    MUL = mybir.AluOpType.mult
    AP = bass.AP

    nc = bacc.Bacc()
    ctx = ExitStack()

    # ---- external IO (all bf16; host pre-casts) ----
    ein = lambda name, shape: nc.dram_tensor(name, shape, bf16, kind="ExternalInput")
    img_xy = ein("img_xy", [CIN, 206])
    img_xz = ein("img_xz", [CIN, 110])
    img_yz = ein("img_yz", [CIN, 614])
    img_tx = ein("img_tx", [CIN, 62])
    img_ty = ein("img_ty", [CIN, 342])
    img_tz = ein("img_tz", [CIN, 182])
    wr = ein("wr", [CIN, 864])
    b_flat = ein("b_flat", [1, 96])
    ones_in = ein("ones_in", [1, 128])
    sel_in = ein("sel_in", [2, 128])
    # out layout: [t, p=(z0,y), f=(x,z1,c)] bf16; host transposes/widens.
    out_d = nc.dram_tensor("out", [T, 128, 2048], bf16, kind="ExternalOutput")

    # ---- DRAM scratch: channels-last conv outputs ----
    dtx = nc.dram_tensor("dtx", [T, XL, C], f32)     # (t, x, c) upsampled
    dtz = nc.dram_tensor("dtz", [T, Z, C], f32)      # (t, z, c) upsampled
    dty = nc.dram_tensor("dty", [8, 64, 16], bf16)   # (t', y, c), y upsampled
    dxy = nc.dram_tensor("dxy", [32, 8, 16], f32)    # (y', x, c), x upsampled
    dxz = nc.dram_tensor("dxz", [XL, Z, C], f32)     # (x, z, c) upsampled
    dyz = nc.dram_tensor("dyz", [32, 32, 16], f32)   # (y', z, c), z upsampled
    # raw conv-output dumps (flat [m*16], junk cols included)
    edump = {}
    for k, m, dt in [("tx", 48, f32), ("tz0", 72, f32), ("tz1", 72, f32),
                     ("xz", 72, f32),
                     ("ty0", 102, bf16), ("ty1", 102, bf16), ("ty2", 68, bf16),
                     ("xy0", 68, f32), ("xy1", 68, f32),
                     ("yz0", 126, f32), ("yz1", 126, f32), ("yz2", 126, f32),
                     ("yz3", 126, f32), ("yz4", 72, f32)]:
        edump[k] = nc.dram_tensor(f"e_{k}", [m * 16], dt)

    with TileContext(nc) as tc:
        sb = lambda name, shape, dt=f32: ctx.enter_context(
            nc.sbuf_tensor(name, shape, dt))
        # inputs (bf16)
        i_xy, i_xz, i_yz = (sb("i_xy", [CIN, 206], bf16),
                            sb("i_xz", [CIN, 110], bf16),
                            sb("i_yz", [CIN, 614], bf16))
        i_tx, i_ty, i_tz = (sb("i_tx", [CIN, 62], bf16),
                            sb("i_ty", [CIN, 342], bf16),
                            sb("i_tz", [CIN, 182], bf16))
        w_sb = sb("w_sb", [CIN, 864], bf16)
        bf_sb = sb("bf_sb", [1, 96], bf16)
        on_sb = sb("on_sb", [1, 128], bf16)
        sel_b = sb("sel_b", [2, 128], bf16)
        # voxel operands
        utx3 = sb("utx3", [32, 128])       # p=(z0,t): (x,c)
        utz3 = sb("utz3", [32, 256])       # p=(z0,t): (z1,c)
        uxza = sb("uxza", [32, 2048])      # p=(z0,t): (x,z1,c), t-dup of xz
        qa = sb("qa", [32, 2048])          # utz*utx
        q32 = sb("q32", [32, 2048], bf16)  # p=(z0,t): (x,z1,c) = Q2
        q_all = sb("q_all", [2, 32768], bf16)   # p=z0: (t,x,z1,c)
        TYs = sb("TYs", [128, 128], bf16)  # p=(z0,y): (t',c)
        uxy = sb("uxy", [128, 128])        # p=(z0,y): (x,c)
        uyz = sb("uyz", [128, 256])        # p=(z0,y): (z1,c)
        m1 = sb("m1", [128, 2048], bf16)   # uxy*uyz

        # ---------- phase A: input loads (Q-path planes first) ----------
        for dst, srca in ((i_tx, img_tx), (i_tz, img_tz), (i_xz, img_xz)):
            nc.sync.dma_start(dst[:], srca[:])
        nc.sync.dma_start(w_sb[:], wr[:])
        nc.sync.dma_start(bf_sb[:], b_flat[:])
        nc.sync.dma_start(on_sb[:], ones_in[:])
        nc.sync.dma_start(sel_b[:], sel_in[:])
        for dst, srca in ((i_ty, img_ty), (i_xy, img_xy), (i_yz, img_yz)):
            nc.sync.dma_start(dst[:], srca[:])

        # ---------- phase B: convolutions + staging ----------
        def wslice(i, dy, dx):
            off = ((i * 3 + dy) * 3 + dx) * 16
            return w_sb[:, off:off + 16]

        conv_pool_cm = tc.tile_pool(name="convpsum", bufs=2, space="PSUM")
        conv_pool = conv_pool_cm.__enter__()

        conv_outs = {}

        def conv_spatial(i, img, fsz, wp, rows, row0, tag, dt=f32):
            # Full-width contiguous windows (stationary AP must be 1-D):
            # out flat m = r*wp + col; junk at cols wp-2, wp-1.
            m = rows * wp
            psum = conv_pool.tile([m, 16], f32, name=f"cp_{tag}", tag="cp")
            for dy in range(3):
                for dx in range(3):
                    lhsT = AP(img, (row0 + dy) * wp + dx, [[fsz, CIN], [1, m]])
                    nc.tensor.matmul(psum, lhsT, wslice(i, dy, dx),
                                     start=(dy == 0 and dx == 0), stop=False)
            nc.tensor.matmul(psum, on_sb[:, :m],
                             bf_sb[:, i * 16:i * 16 + 16], start=False, stop=True)
            out_sb = sb(f"c_{tag}", [m, 16], dt)
            nc.scalar.activation(out_sb[:], psum, AF.Relu)
            conv_outs[tag] = out_sb

        dump_insts = {}
        stage_insts = {}

        def dump(eng, k):
            dump_insts[k] = eng.dma_start(edump[k][:], conv_outs[k][:])

        def stage(eng, key, srck, dst_ap, src_ap):
            inst = eng.dma_start(dst_ap, src_ap)
            if srck is not None:
                bass._add_dep_helper(inst.ins, dump_insts[srck].ins,
                                     reason=f"dump {srck}")
            stage_insts.setdefault(key, []).append(inst)

        def after_stage(key, inst):
            for si in stage_insts[key]:
                bass._add_dep_helper(inst.ins, si.ins, reason=f"raw {key}")
            return inst

        # --- tx ---
        conv_spatial(3, i_tx, 62, 6, 8, 0, "tx")              # m=48
        dump(nc.sync, "tx")
        for rt in range(2):
            for rx in range(2):
                stage(nc.sync, "dtx", "tx",
                      AP(dtx, rt * XL * C + rx * C,
                         [[2 * XL * C, 8], [2 * C, 4], [1, 16]]),
                      AP(edump["tx"], 0, [[96, 8], [16, 4], [1, 16]]))
        # --- tz ---
        conv_spatial(5, i_tz, 182, 18, 4, 0, "tz0")           # m=72
        conv_spatial(5, i_tz, 182, 18, 4, 4, "tz1")
        for k in range(2):
            dump(nc.sync, f"tz{k}")
            for rt in range(2):
                for rz in range(2):
                    stage(nc.sync, "dtz", f"tz{k}",
                          AP(dtz, (8 * k + rt) * Z * C + rz * C,
                             [[2 * Z * C, 4], [2 * C, 16], [1, 16]]),
                          AP(edump[f"tz{k}"], 0, [[288, 4], [16, 16], [1, 16]]))
        # --- xz (q path now: folded into Q2) ---
        conv_spatial(1, i_xz, 110, 18, 4, 0, "xz")            # m=72
        dump(nc.sync, "xz")
        for rx in range(2):
            for rz in range(2):
                stage(nc.sync, "dxz", "xz",
                      AP(dxz, rx * Z * C + rz * C,
                         [[2 * Z * C, 4], [2 * C, 16], [1, 16]]),
                      AP(edump["xz"], 0, [[288, 4], [16, 16], [1, 16]]))

        # --- Q2 = utx * utz * uxz  -> q_all[z0, (t,x,z1,c)] ---
        for z0 in range(2):
            after_stage("dtx", nc.sync.dma_start(
                AP(utx3, z0 * 16 * 128, [[128, 16], [1, 128]]),
                AP(dtx, 0, [[128, 16], [1, 128]])))
            after_stage("dtz", nc.sync.dma_start(
                AP(utz3, z0 * 16 * 256, [[256, 16], [1, 256]]),
                AP(dtz, z0 * 256, [[512, 16], [1, 256]])))
            after_stage("dxz", nc.sync.dma_start(
                AP(uxza, z0 * 16 * 2048, [[2048, 16], [1, 2048]]),
                AP(dxz, z0 * 256, [[0, 16], [512, 8], [1, 256]])))
        nc.vector.tensor_tensor(
            qa[:], AP(utz3, 0, [[256, 32], [0, 8], [1, 256]]),
            AP(utx3, 0, [[128, 32], [16, 8], [0, 16], [1, 16]]), MUL)
        nc.vector.tensor_tensor(q32[:], qa[:], uxza[:], MUL)
        nc.sync.dma_start(
            AP(q_all, 0, [[32768, 2], [2048, 16], [1, 2048]]), q32[:])

        # --- ty (position-partition conv; (t',y,c) bf16, y-up in staging) ---
        ty_rows = [(0, 3), (3, 3), (6, 2)]
        for bb, (r0, nr) in enumerate(ty_rows):
            conv_spatial(4, i_ty, 342, 34, nr, r0, f"ty{bb}", dt=bf16)
            dump(nc.scalar, f"ty{bb}")
            for yd in range(2):
                stage(nc.scalar, "dty", f"ty{bb}",
                      AP(dty, r0 * 1024 + yd * 16,
                         [[1024, nr], [32, 32], [1, 16]]),
                      AP(edump[f"ty{bb}"], 0, [[544, nr], [16, 32], [1, 16]]))
        for z0 in range(2):
            after_stage("dty", nc.scalar.dma_start(
                AP(TYs, z0 * 64 * 128, [[128, 64], [1, 128]]),
                AP(dty, 0, [[16, 64], [1024, 8], [1, 16]])))

        # --- xy (compact y'; x upsampled in staging) ---
        conv_spatial(0, i_xy, 206, 34, 2, 0, "xy0")           # m=68
        conv_spatial(0, i_xy, 206, 34, 2, 2, "xy1")
        for k in range(2):
            dump(nc.scalar, f"xy{k}")
            for xd in range(2):
                stage(nc.scalar, "dxy", f"xy{k}",
                      AP(dxy, (4 * k + xd) * 16,
                         [[128, 32], [32, 2], [1, 16]]),
                      AP(edump[f"xy{k}"], 0, [[16, 32], [544, 2], [1, 16]]))
        # --- yz (compact y'; z upsampled in staging) ---
        yz_rows = [(0, 7), (7, 7), (14, 7), (21, 7), (28, 4)]
        for bb, (r0, nr) in enumerate(yz_rows):
            conv_spatial(2, i_yz, 614, 18, nr, r0, f"yz{bb}")
            eng = nc.sync if bb % 2 else nc.scalar
            dump(eng, f"yz{bb}")
            for zd in range(2):
                stage(eng, "dyz", f"yz{bb}",
                      AP(dyz, r0 * 512 + zd * 16,
                         [[512, nr], [32, 16], [1, 16]]),
                      AP(edump[f"yz{bb}"], 0, [[288, nr], [16, 16], [1, 16]]))

        conv_pool_cm.__exit__(None, None, None)

        # ---------- M1' = uxy * uyz (bf16) ----------
        for z0 in range(2):
            after_stage("dxy", nc.scalar.dma_start(
                AP(uxy, z0 * 64 * 128, [[128, 64], [1, 128]]),
                AP(dxy, 0, [[128, 32], [0, 2], [1, 128]])))
            after_stage("dyz", nc.scalar.dma_start(
                AP(uyz, z0 * 64 * 256, [[256, 64], [1, 256]]),
                AP(dyz, z0 * 256, [[512, 32], [0, 2], [1, 256]])))
        nc.vector.tensor_tensor(
            m1[:], AP(uyz, 0, [[256, 128], [0, 8], [1, 256]]),
            AP(uxy, 0, [[128, 128], [16, 8], [0, 16], [1, 16]]), MUL)

        # ---------- phase E: per-t voxel ----------
        from contextlib import ExitStack as _ES
        pool_ctx = _ES()
        v_pool = pool_ctx.enter_context(
            tc.tile_pool(name="vps", bufs=2, space="PSUM"))
        qbc_pool = pool_ctx.enter_context(tc.tile_pool(name="qbc", bufs=3))
        tq_pool = pool_ctx.enter_context(tc.tile_pool(name="tq", bufs=3))
        out_pool = pool_ctx.enter_context(tc.tile_pool(name="outsb", bufs=3))

        for t in range(T):
            v = v_pool.tile([128, 2048], f32, name="v", tag="v")
            for bank in range(4):
                rhs = AP(q_all, t * 2048 + bank * 512, [[32768, 2], [1, 512]])
                nc.tensor.matmul(v[:, bank * 512:(bank + 1) * 512],
                                 sel_b[:], rhs, start=True, stop=True)
            qbc = qbc_pool.tile([128, 2048], bf16, name="qbc", tag="qbc")
            nc.scalar.activation(qbc[:], v[:], AF.Copy)
            tq = tq_pool.tile([128, 2048], bf16, name="tq", tag="tq")
            nc.vector.tensor_tensor(
                tq[:], qbc[:],
                AP(TYs, (t // 2) * 16, [[128, 128], [0, 8], [0, 16], [1, 16]]),
                MUL)
            o = out_pool.tile([128, 2048], bf16, name="o", tag="o")
            nc.vector.tensor_tensor(o[:], m1[:], tq[:], MUL)
            nc.sync.dma_start(
                AP(out_d, t * 128 * 2048, [[2048, 128], [1, 2048]]), o[:])

        pool_ctx.close()

    nc.compile()
    return nc, ctx


def _prep_inputs(plane_xy, plane_xz, plane_yz, plane_tx, plane_ty, plane_tz, W, b):
    """Host-side slicing/padding/transposition. Returns per-core input maps."""
    import ml_dtypes
    f32 = np.float32
    bf = ml_dtypes.bfloat16
    xy = np.asarray(plane_xy, f32)[0]  # [64, X'32, Y'32]
    xz = np.asarray(plane_xz, f32)[0]  # [64, X'32, Z'16]
    yz = np.asarray(plane_yz, f32)[0]  # [64, Y'32, Z'16]
    tx = np.asarray(plane_tx, f32)[0]  # [64, T'8,  X'32]
    ty = np.asarray(plane_ty, f32)[0]  # [64, T'8,  Y'32]
    tz = np.asarray(plane_tz, f32)[0]  # [64, T'8,  Z'16]
    W = np.asarray(W, f32)             # [6, 16, 64, 3, 3]
    b = np.asarray(b, f32)             # [6, 16]

    wr = np.ascontiguousarray(
        W.transpose(2, 0, 3, 4, 1).reshape(CIN, 864)).astype(bf)
    b_flat = np.ascontiguousarray(b.reshape(1, 96)).astype(bf)
    ones = np.ones((1, 128), bf)
    sel = np.zeros((2, 128), f32)
    sel[0, :64] = 1.0
    sel[1, 64:] = 1.0
    sel = sel.astype(bf)

    def flat2(p):
        q = p.reshape(p.shape[0], -1)
        return np.ascontiguousarray(np.pad(q, ((0, 0), (0, 2)))).astype(bf)

    img_yz = flat2(np.pad(yz, ((0, 0), (1, 1), (1, 1))))
    img_ty = flat2(np.pad(ty, ((0, 0), (1, 1), (1, 1))))
    img_tz = flat2(np.pad(tz, ((0, 0), (1, 1), (1, 1))))

    def row_halo(p, x0h):
        out = np.zeros((p.shape[0], 6, p.shape[2]), f32)
        lo = x0h - 1
        s0, s1 = max(lo, 0), min(lo + 6, p.shape[1])
        out[:, s0 - lo:s0 - lo + (s1 - s0), :] = p[:, s0:s1, :]
        return out

    def col_halo(p, x0h):
        out = np.zeros((p.shape[0], p.shape[1], 6), f32)
        lo = x0h - 1
        s0, s1 = max(lo, 0), min(lo + 6, p.shape[2])
        out[:, :, s0 - lo:s0 - lo + (s1 - s0)] = p[:, :, s0:s1]
        return out

    in_maps = []
    for k in range(NCORES):
        x0h = 4 * k
        in_maps.append({
            "img_xy": flat2(np.pad(row_halo(xy, x0h), ((0, 0), (0, 0), (1, 1)))),
            "img_xz": flat2(np.pad(row_halo(xz, x0h), ((0, 0), (0, 0), (1, 1)))),
            "img_yz": img_yz,
            "img_tx": flat2(np.pad(col_halo(tx, x0h), ((0, 0), (1, 1), (0, 0)))),
            "img_ty": img_ty,
            "img_tz": img_tz,
            "wr": wr,
            "b_flat": b_flat,
            "ones_in": ones,
            "sel_in": sel,
        })
    return in_maps


def _unshard(out_np):
    # device out: [T, p=(z0,y), f=(x,z1,c)] -> [T, XL, Y, Z, C] f32
    o = np.asarray(out_np).reshape(T, 2, Y, XL, 16, C)
    o = o.transpose(0, 3, 2, 1, 4, 5).reshape(T, XL, Y, Z, C)
    return o.astype(np.float32)


def kernel(plane_xy, plane_xz, plane_yz, plane_tx, plane_ty, plane_tz, W, b):
    from concourse.bass_utils import run_bass_kernel_spmd

    if "nc" not in _CACHE:
        _CACHE["nc"], _CACHE["ctx"] = _build_program()
    nc = _CACHE["nc"]

    in_maps = _prep_inputs(plane_xy, plane_xz, plane_yz, plane_tx, plane_ty,
                           plane_tz, W, b)
    res = run_bass_kernel_spmd(nc, in_maps, list(range(NCORES)))
    slices = [_unshard(res.results[k]["out"]) for k in range(NCORES)]
    full = np.concatenate(slices, axis=1)  # [T, 64, Y, Z, C]
    return full[None].astype(np.float32)
